# revision 28
# baseline (speedup 1.0000x reference)
"""CRF Viterbi decode (tf.contrib.crf.crf_decode equivalent) on 8 Trainium2 cores.

Problem: potentials [2048, 512, 64] f32, transitions [64, 64] f32,
sequence_length [2048, 1] i32 -> (tags [2048, 512] i32, best_score [2048] f32).

Strategy:
 - Data-parallel over batch: 16 tiles of 128 batches, sorted by sequence
   length (descending); core c owns tiles (c, 15-c) so per-core work is
   balanced and each tile's time loop stops at that tile's max length.
 - Forward (per tile, per step t): one fused DVE pass builds the [128, 64x64]
   candidate matrix cand[b, j, i] = alpha[b, i] + trans[i, j] via a step-0
   broadcast AP, a grouped reduce-max produces new_alpha[b, j]; emissions are
   added and a predicated copy freezes batches with t >= L. The running alpha
   state is streamed to a DRAM history buffer.
 - Backward: tags are reconstructed by re-running the argmax for only the
   winning column per step: a one-hot of the current tag is built on DVE,
   transposed on the PE, and a PE matmul gathers trans[:, tag]; argmax with
   first-index tie-breaking uses an is_equal * (64-i) encode + reduce-max
   (bit-exact against jnp.argmax).
"""

import numpy as np

B, T, K = 2048, 512, 64
P = 128
NCORES = 8
NTILES = B // P  # 16
TPC = NTILES // NCORES  # tiles per core = 2

_CACHE = {}


def _patch_tile_drain():
    """The walrus build in this container allows only ONE sync wait on CTRL
    (NoOp/Drain) instructions; TileContext's kernel-tail drain attaches one
    wait per live semaphore. Split them across single-wait nops."""
    import concourse.tile as tile
    from concourse.vector_clock import ScopedClock, VectorClock

    if getattr(tile.TileContext, "_drain_patch", False):
        return

    def _drain_and_barrier(self, tick_clock, wait_clock):
        gc = tick_clock.global_clock
        ticks = []
        for i in range(256):
            try:
                ticks.append(gc.peek_next(i) - 1)
            except Exception:
                break
        for i, t in enumerate(ticks):
            if t <= 0:
                continue
            vec = [0] * len(ticks)
            vec[i] = t
            ni = self.nc.sync.nop(nofuse=True, hint=f"drain_w{i}").ins
            wait_clock.add_sem_waits(ni, ScopedClock({None: VectorClock(vec)}))
        self.nc.sync.drain()
        self.nc.all_engine_barrier()
        assert self.sems is not None
        popped = self.nc._tile_sem_poison_stack.pop()
        assert popped is self._sem_poison
        self.nc.clear_and_free_semaphores(list(self.sems.allocated().values()))
        self.nc.all_engine_barrier()

    tile.TileContext._drain_and_barrier = _drain_and_barrier
    tile.TileContext._drain_patch = True


def _spill_excess_waits(nc):
    """This container's walrus accepts at most ONE sync wait (and update) per
    instruction; Tile's scheduler can attach several. Rewrite every block,
    moving excess waits onto single-wait NoOps inserted just before the
    instruction (same engine), and excess updates onto NoOps just after."""
    import bass_rust

    cnt = 0
    for f in nc.m.functions:
        for bb in f.blocks:
            insts = list(bb.instructions)
            need = False
            for ins in insts:
                si = ins.sync_info
                if type(ins).__name__ == "InstSeqAssert" or (
                    si and ((si.on_wait and len(si.on_wait) > 1)
                            or (si.on_update and len(si.on_update) > 1))):
                    need = True
                    break
            if not need:
                continue
            out = []
            for ins in insts:
                si = ins.sync_info
                waits = list(si.on_wait) if si and si.on_wait else []
                ups = list(si.on_update) if si and si.on_update else []
                if type(ins).__name__ == "InstSeqAssert":
                    # runtime bounds asserts serialize as pseudo-ISA ops the
                    # public walrus can't encode; replace with a NoOp that
                    # keeps any sync side effects
                    cnt += 1
                    n = bass_rust.InstNoOp(name=f"I-spa{cnt}", ins=[], outs=[])
                    n.engine = ins.engine
                    n.sync_info = bass_rust.SyncInfo(on_wait=waits[:1],
                                                     on_update=ups[:1])
                    out.append(n)
                    continue
                for w in waits[:-1]:
                    cnt += 1
                    n = bass_rust.InstNoOp(name=f"I-spw{cnt}", ins=[], outs=[])
                    n.engine = ins.engine
                    n.sync_info = bass_rust.SyncInfo(on_wait=[w], on_update=[])
                    out.append(n)
                if len(waits) > 1 or len(ups) > 1:
                    ins.sync_info = bass_rust.SyncInfo(
                        on_wait=waits[-1:], on_update=ups[:1])
                out.append(ins)
                for u in ups[1:]:
                    cnt += 1
                    n = bass_rust.InstNoOp(name=f"I-spu{cnt}", ins=[], outs=[])
                    n.engine = ins.engine
                    n.sync_info = bass_rust.SyncInfo(on_wait=[], on_update=[u])
                    out.append(n)
            bb.instructions = out
    return cnt


def _build_program():
    import sys
    try:
        import concourse.bass as bass  # noqa
    except ImportError:
        sys.path.insert(0, "/opt/trn_rl_repo")
    import concourse.bass as bass
    import concourse.mybir as mybir
    from concourse.tile import TileContext
    from concourse.bass import ds

    _patch_tile_drain()

    f32 = mybir.dt.float32
    i32 = mybir.dt.int32
    Op = mybir.AluOpType

    nc = bass.Bass()

    d_pot = nc.dram_tensor("pot", [TPC, P, T + 1, K], f32, kind="ExternalInput")
    u8 = mybir.dt.uint8
    d_mask = nc.dram_tensor("maskf", [TPC, P, T + 1], u8, kind="ExternalInput")
    d_meta = nc.dram_tensor("meta", [1, 2 * TPC], i32, kind="ExternalInput")
    d_transCR = nc.dram_tensor("transCR", [P, K * K], f32, kind="ExternalInput")
    d_transT = nc.dram_tensor("transT", [K, K], f32, kind="ExternalInput")
    d_ident = nc.dram_tensor("ident", [P, P], f32, kind="ExternalInput")
    d_rev = nc.dram_tensor("rev64", [P, K], f32, kind="ExternalInput")
    d_tags = nc.dram_tensor("o_tags", [TPC, P, T], i32, kind="ExternalOutput")
    d_best = nc.dram_tensor("o_best", [TPC, P, 1], f32, kind="ExternalOutput")
    d_hist = nc.dram_tensor("hist", [TPC, T + 1, P, K], f32)  # internal scratch

    with TileContext(nc) as tc:
        with (
            tc.tile_pool(name="const", bufs=1) as cpool,
            tc.tile_pool(name="state", bufs=1) as spool,
            tc.tile_pool(name="io", bufs=2) as iop,
            tc.tile_pool(name="psum", bufs=2, space="PSUM") as pp,
        ):
            t_transCR = cpool.tile([P, K * K], f32)
            t_transT = cpool.tile([K, K], f32)
            t_ident = cpool.tile([P, P], f32)
            t_rev = cpool.tile([P, K], f32)
            t_meta = cpool.tile([1, 2 * TPC], i32)
            nc.sync.dma_start(t_transCR, d_transCR[:])
            nc.sync.dma_start(t_transT, d_transT[:])
            nc.sync.dma_start(t_ident, d_ident[:])
            nc.sync.dma_start(t_rev, d_rev[:])
            nc.sync.dma_start(t_meta, d_meta[:])

            def _loadsc(name, col, lo, hi):
                regs = nc.alloc_registers(name)
                for h in regs.handles:
                    nc.engines[h.engine].reg_load(h, t_meta[0:1, col : col + 1])
                return nc.snap(regs, min_val=lo, max_val=hi)

            g2s = [_loadsc(f"g2{k}", 2 * k, 0, T) for k in range(TPC)]
            lmaxs = [_loadsc(f"lmax{k}", 2 * k + 1, 1, T) for k in range(TPC)]

            for tile in range(TPC):
                lmax = lmaxs[tile]
                t_alpha = spool.tile([P, K], f32, tag=f"alpha{tile}")
                t_mask = spool.tile([P, T + 1], u8, tag=f"mask{tile}")
                t_tags = spool.tile([P, T], f32, tag=f"tags{tile}")
                nc.sync.dma_start(t_alpha, d_pot[tile, :, 0, :])
                nc.sync.dma_start(t_mask, d_mask[tile])
                nc.sync.dma_start(d_hist[tile, 0], t_alpha)

                # ---------------- forward ----------------
                # 2-step unrolled groups; GPSIMD (otherwise idle) adds
                # j-blocks [H, K) while DVE adds [0, H) and reduces both.
                # Chunked emit load + hist store (one dynamic DMA each per
                # group) keep SP bounds-check registers within budget.
                H = K // 2
                trCR3 = t_transCR.rearrange("p (j i) -> p j i", i=K)

                def fwd_step(t, t_emit, stage_slot, t_alpha=t_alpha,
                             t_mask=t_mask):
                    t_cand = iop.tile([P, K * K], f32, tag="cand")
                    cand3 = t_cand.rearrange("p (j i) -> p j i", i=K)
                    nc.vector.tensor_tensor(
                        out=cand3, in0=trCR3,
                        in1=t_alpha[:, None, :].broadcast_to([P, K, K]),
                        op=Op.add,
                    )
                    t_new = iop.tile([P, K], f32, tag="new")
                    nc.vector.tensor_reduce(
                        out=t_new, in_=cand3, axis=mybir.AxisListType.X, op=Op.max
                    )
                    nc.vector.tensor_tensor(out=t_new, in0=t_new, in1=t_emit, op=Op.add)
                    nc.vector.copy_predicated(
                        t_alpha, t_mask[:, ds(t, 1)].broadcast_to([P, K]), t_new
                    )
                    nc.vector.tensor_copy(stage_slot, t_alpha)

                g2 = g2s[tile]
                with tc.For_i(0, g2, 2, name=f"fwd2_{tile}", staggered_reset=True) as i2:
                    t0 = nc.s_assert_within(i2 + 1, 1, T - 1,
                                            skip_runtime_assert=True)
                    t_emitC = iop.tile([P, 2 * K], f32, tag="emitC")
                    nc.sync.dma_start(
                        t_emitC.rearrange("p (u k) -> p u k", k=K),
                        d_pot[tile][:, ds(t0, 2), :])
                    t_stage = iop.tile([P, 2 * K], f32, tag="stage")
                    for u in range(2):
                        tu = nc.s_assert_within(i2 + (1 + u), 1, T,
                                                skip_runtime_assert=True)
                        fwd_step(tu, t_emitC[:, u * K:(u + 1) * K],
                                 t_stage[:, u * K:(u + 1) * K])
                    nc.sync.dma_start(
                        d_hist[tile][ds(t0, 2), :, :].transpose([1, 0, 2]),
                        t_stage.rearrange("p (u k) -> p u k", k=K))

                # ---------------- final scores / last tag ----------------
                t_best = spool.tile([P, 1], f32, tag=f"best{tile}")
                nc.vector.tensor_reduce(
                    out=t_best, in_=t_alpha, axis=mybir.AxisListType.X, op=Op.max
                )
                t_encf = iop.tile([P, K], f32, tag="encf")
                nc.vector.scalar_tensor_tensor(
                    out=t_encf, in0=t_alpha, scalar=t_best, in1=t_rev,
                    op0=Op.is_equal, op1=Op.mult,
                )
                t_lt = spool.tile([P, 1], f32, tag=f"lt{tile}")
                nc.vector.tensor_reduce(
                    out=t_lt, in_=t_encf, axis=mybir.AxisListType.X, op=Op.max
                )
                nc.sync.dma_start(d_best[tile], t_best)
                # prefill all tag columns with the (encoded) last tag
                nc.vector.tensor_copy(t_tags, t_lt.broadcast_to([P, T]))

                # ---------------- backward ----------------
                with tc.For_i(1, lmax, staggered_reset=True) as i:
                    t = nc.s_assert_within(lmax - i, 1, T - 1)
                    t_hist = iop.tile([P, K], f32, tag="hist")
                    nc.sync.dma_start(t_hist, d_hist[tile][ds(t - 1, 1), :, :])
                    # one-hot of current tag: rev64[p,i] == enc  <=>  i == tag
                    t_selB = iop.tile([P, K], f32, tag="selB")
                    nc.vector.tensor_tensor(
                        out=t_selB, in0=t_rev,
                        in1=t_tags[:, ds(t, 1)].broadcast_to([P, K]),
                        op=Op.is_equal,
                    )
                    ps_selT = pp.tile([K, P], f32, tag="selT")
                    nc.tensor.transpose(ps_selT, t_selB, t_ident)
                    t_selT = iop.tile([K, P], f32, tag="selTs")
                    nc.scalar.copy(t_selT, ps_selT)
                    ps_gat = pp.tile([P, K], f32, tag="gat")
                    nc.tensor.matmul(
                        ps_gat, lhsT=t_selT, rhs=t_transT, start=True, stop=True
                    )
                    t_candb = iop.tile([P, K], f32, tag="candb")
                    nc.vector.tensor_tensor(
                        out=t_candb, in0=t_hist, in1=ps_gat, op=Op.add
                    )
                    t_bb = iop.tile([P, 1], f32, tag="bb")
                    nc.vector.tensor_reduce(
                        out=t_bb, in_=t_candb, axis=mybir.AxisListType.X, op=Op.max
                    )
                    t_encb = iop.tile([P, K], f32, tag="encb")
                    nc.vector.scalar_tensor_tensor(
                        out=t_encb, in0=t_candb, scalar=t_bb, in1=t_rev,
                        op0=Op.is_equal, op1=Op.mult,
                    )
                    t_penc = iop.tile([P, 1], f32, tag="penc")
                    nc.vector.tensor_reduce(
                        out=t_penc, in_=t_encb, axis=mybir.AxisListType.X, op=Op.max
                    )
                    nc.vector.copy_predicated(
                        t_tags[:, ds(t - 1, 1)], t_mask[:, ds(t, 1)], t_penc
                    )

                # decode: tag = 64 - enc, cast to int32
                t_tagsi = spool.tile([P, T], i32, tag=f"tagsi{tile}")
                nc.vector.tensor_scalar(
                    out=t_tagsi, in0=t_tags, scalar1=-1.0, scalar2=64.0,
                    op0=Op.mult, op1=Op.add,
                )
                nc.sync.dma_start(d_tags[tile], t_tagsi)

    return nc


def _prep_inputs(potentials, transitions, sequence_length):
    potentials = np.ascontiguousarray(np.asarray(potentials, dtype=np.float32))
    transitions = np.ascontiguousarray(np.asarray(transitions, dtype=np.float32))
    L = np.asarray(sequence_length).reshape(-1).astype(np.int64)

    order = np.argsort(-L, kind="stable")
    tiles = order.reshape(NTILES, P)
    Lmax_tile = L[tiles].max(axis=1).astype(np.int32)

    transT = np.ascontiguousarray(transitions.T)
    transCR = np.tile(transT.reshape(1, K * K), (P, 1)).astype(np.float32)
    ident = np.eye(P, dtype=np.float32)
    rev64 = np.tile((64.0 - np.arange(K, dtype=np.float32)).reshape(1, K), (P, 1))

    tvec = np.arange(T, dtype=np.int64)
    in_maps = []
    core_tiles = []
    for c in range(NCORES):
        ta, tb = c, NTILES - 1 - c
        sel = np.concatenate([tiles[ta], tiles[tb]])
        core_tiles.append((tiles[ta], tiles[tb]))
        pot_c = np.zeros((TPC, P, T + 1, K), dtype=np.float32)
        pot_c[:, :, :T, :] = potentials[sel].reshape(TPC, P, T, K)
        tvec1 = np.arange(T + 1, dtype=np.int64)
        mask_c = (tvec1[None, :] < np.minimum(L[sel][:, None], T)
                  ).astype(np.uint8).reshape(TPC, P, T + 1)
        def _g2(lm):
            return -((1 - int(lm)) // 2) * 2  # ceil((lm-1)/2)*2
        meta_c = np.array([[_g2(Lmax_tile[ta]), Lmax_tile[ta],
                            _g2(Lmax_tile[tb]), Lmax_tile[tb]]], dtype=np.int32)
        in_maps.append({
            "pot": pot_c,
            "maskf": np.ascontiguousarray(mask_c),
            "meta": meta_c,
            "transCR": transCR,
            "transT": transT,
            "ident": ident,
            "rev64": rev64,
        })
    return in_maps, core_tiles


def _run_hw(nc, in_maps, trace=False):
    from concourse.bass_utils import run_bass_kernel_spmd
    if not _CACHE.get("spilled"):
        _spill_excess_waits(nc)
        _CACHE["spilled"] = True
    res = run_bass_kernel_spmd(nc, in_maps, list(range(NCORES)), trace=trace)
    return res


def _run_sim(nc, in_maps, cores=None):
    from concourse import bass_interp
    outs = []
    for ci, im in enumerate(in_maps):
        if cores is not None and ci not in cores:
            outs.append(None)
            continue
        sim = bass_interp.CoreSim(nc)
        for k, v in im.items():
            sim.tensor(k)[:] = v
        sim.simulate()
        outs.append({
            "o_tags": np.array(sim.tensor("o_tags")),
            "o_best": np.array(sim.tensor("o_best")),
        })
    return outs


def kernel(potentials, transitions, sequence_length, _mode="hw", _cores=None,
           _trace=False):
    nc = _CACHE.get("nc")
    if nc is None:
        nc = _build_program()
        _CACHE["nc"] = nc
    in_maps, core_tiles = _prep_inputs(potentials, transitions, sequence_length)

    if _mode == "sim":
        results = _run_sim(nc, in_maps, cores=_cores)
    else:
        bres = _run_hw(nc, in_maps, trace=_trace)
        _CACHE["last_bres"] = bres
        results = bres.results

    tags = np.zeros((B, T), dtype=np.int32)
    best = np.zeros((B,), dtype=np.float32)
    for c, (rows_a, rows_b) in enumerate(core_tiles):
        r = results[c]
        if r is None:
            continue
        o_tags = np.asarray(r["o_tags"]).reshape(TPC, P, T)
        o_best = np.asarray(r["o_best"]).reshape(TPC, P)
        tags[rows_a] = o_tags[0]
        tags[rows_b] = o_tags[1]
        best[rows_a] = o_best[0]
        best[rows_b] = o_best[1]
    return tags, best


# revision 30
# speedup vs baseline: 1.1626x; 1.1626x over previous
"""CRF Viterbi decode (tf.contrib.crf.crf_decode equivalent) on 8 Trainium2 cores.

Problem: potentials [2048, 512, 64] f32, transitions [64, 64] f32,
sequence_length [2048, 1] i32 -> (tags [2048, 512] i32, best_score [2048] f32).

Strategy:
 - Data-parallel over batch: 16 tiles of 128 batches, sorted by sequence
   length (descending); core c owns tiles (c, 15-c) so per-core work is
   balanced and each tile's time loop stops at that tile's max length.
 - Forward (per tile, per step t): one fused DVE pass builds the [128, 64x64]
   candidate matrix cand[b, j, i] = alpha[b, i] + trans[i, j] via a step-0
   broadcast AP, a grouped reduce-max produces new_alpha[b, j]; emissions are
   added and a predicated copy freezes batches with t >= L. The running alpha
   state is streamed to a DRAM history buffer.
 - Backward: tags are reconstructed by re-running the argmax for only the
   winning column per step: a one-hot of the current tag is built on DVE,
   transposed on the PE, and a PE matmul gathers trans[:, tag]; argmax with
   first-index tie-breaking uses an is_equal * (64-i) encode + reduce-max
   (bit-exact against jnp.argmax).
"""

import numpy as np

B, T, K = 2048, 512, 64
P = 128
NCORES = 8
NTILES = B // P  # 16
TPC = NTILES // NCORES  # tiles per core = 2

_CACHE = {}


def _patch_tile_drain():
    """The walrus build in this container allows only ONE sync wait on CTRL
    (NoOp/Drain) instructions; TileContext's kernel-tail drain attaches one
    wait per live semaphore. Split them across single-wait nops."""
    import concourse.tile as tile
    from concourse.vector_clock import ScopedClock, VectorClock

    if getattr(tile.TileContext, "_drain_patch", False):
        return

    def _drain_and_barrier(self, tick_clock, wait_clock):
        gc = tick_clock.global_clock
        ticks = []
        for i in range(256):
            try:
                ticks.append(gc.peek_next(i) - 1)
            except Exception:
                break
        for i, t in enumerate(ticks):
            if t <= 0:
                continue
            vec = [0] * len(ticks)
            vec[i] = t
            ni = self.nc.sync.nop(nofuse=True, hint=f"drain_w{i}").ins
            wait_clock.add_sem_waits(ni, ScopedClock({None: VectorClock(vec)}))
        self.nc.sync.drain()
        self.nc.all_engine_barrier()
        assert self.sems is not None
        popped = self.nc._tile_sem_poison_stack.pop()
        assert popped is self._sem_poison
        self.nc.clear_and_free_semaphores(list(self.sems.allocated().values()))
        self.nc.all_engine_barrier()

    tile.TileContext._drain_and_barrier = _drain_and_barrier
    tile.TileContext._drain_patch = True


def _spill_excess_waits(nc):
    """This container's walrus accepts at most ONE sync wait (and update) per
    instruction; Tile's scheduler can attach several. Rewrite every block,
    moving excess waits onto single-wait NoOps inserted just before the
    instruction (same engine), and excess updates onto NoOps just after."""
    import bass_rust

    cnt = 0
    for f in nc.m.functions:
        for bb in f.blocks:
            insts = list(bb.instructions)
            need = False
            for ins in insts:
                si = ins.sync_info
                if type(ins).__name__ == "InstSeqAssert" or (
                    si and ((si.on_wait and len(si.on_wait) > 1)
                            or (si.on_update and len(si.on_update) > 1))):
                    need = True
                    break
            if not need:
                continue
            out = []
            for ins in insts:
                si = ins.sync_info
                waits = list(si.on_wait) if si and si.on_wait else []
                ups = list(si.on_update) if si and si.on_update else []
                if type(ins).__name__ == "InstSeqAssert":
                    # runtime bounds asserts serialize as pseudo-ISA ops the
                    # public walrus can't encode; replace with a NoOp that
                    # keeps any sync side effects
                    cnt += 1
                    n = bass_rust.InstNoOp(name=f"I-spa{cnt}", ins=[], outs=[])
                    n.engine = ins.engine
                    n.sync_info = bass_rust.SyncInfo(on_wait=waits[:1],
                                                     on_update=ups[:1])
                    out.append(n)
                    continue
                for w in waits[:-1]:
                    cnt += 1
                    n = bass_rust.InstNoOp(name=f"I-spw{cnt}", ins=[], outs=[])
                    n.engine = ins.engine
                    n.sync_info = bass_rust.SyncInfo(on_wait=[w], on_update=[])
                    out.append(n)
                if len(waits) > 1 or len(ups) > 1:
                    ins.sync_info = bass_rust.SyncInfo(
                        on_wait=waits[-1:], on_update=ups[:1])
                out.append(ins)
                for u in ups[1:]:
                    cnt += 1
                    n = bass_rust.InstNoOp(name=f"I-spu{cnt}", ins=[], outs=[])
                    n.engine = ins.engine
                    n.sync_info = bass_rust.SyncInfo(on_wait=[], on_update=[u])
                    out.append(n)
            bb.instructions = out
    return cnt


def _build_program():
    import sys
    try:
        import concourse.bass as bass  # noqa
    except ImportError:
        sys.path.insert(0, "/opt/trn_rl_repo")
    import concourse.bass as bass
    import concourse.mybir as mybir
    from concourse.tile import TileContext
    from concourse.bass import ds

    _patch_tile_drain()

    f32 = mybir.dt.float32
    i32 = mybir.dt.int32
    Op = mybir.AluOpType

    nc = bass.Bass()

    d_pot = nc.dram_tensor("pot", [TPC, P, T + 1, K], f32, kind="ExternalInput")
    u8 = mybir.dt.uint8
    d_mask = nc.dram_tensor("maskf", [TPC, P, T + 1], u8, kind="ExternalInput")
    d_meta = nc.dram_tensor("meta", [1, 2 * TPC], i32, kind="ExternalInput")
    d_transCR = nc.dram_tensor("transCR", [P, K * K], f32, kind="ExternalInput")
    d_transT = nc.dram_tensor("transT", [K, K], f32, kind="ExternalInput")
    d_ident = nc.dram_tensor("ident", [P, P], f32, kind="ExternalInput")
    d_rev = nc.dram_tensor("rev64", [P, K], f32, kind="ExternalInput")
    d_tags = nc.dram_tensor("o_tags", [TPC, P, T], i32, kind="ExternalOutput")
    d_best = nc.dram_tensor("o_best", [TPC, P, 1], f32, kind="ExternalOutput")
    d_hist = nc.dram_tensor("hist", [TPC, T + 1, P, K], f32)  # internal scratch

    with TileContext(nc) as tc:
        with (
            tc.tile_pool(name="const", bufs=1) as cpool,
            tc.tile_pool(name="state", bufs=1) as spool,
            tc.tile_pool(name="io", bufs=2) as iop,
            tc.tile_pool(name="psum", bufs=2, space="PSUM") as pp,
        ):
            t_transCR = cpool.tile([P, K * K], f32)
            t_transT = cpool.tile([K, K], f32)
            t_ident = cpool.tile([P, P], f32)
            t_rev = cpool.tile([P, K], f32)
            t_meta = cpool.tile([1, 2 * TPC], i32)
            nc.sync.dma_start(t_transCR, d_transCR[:])
            nc.sync.dma_start(t_transT, d_transT[:])
            nc.sync.dma_start(t_ident, d_ident[:])
            nc.sync.dma_start(t_rev, d_rev[:])
            nc.sync.dma_start(t_meta, d_meta[:])

            def _loadsc(name, col, lo, hi):
                regs = nc.alloc_registers(name)
                for h in regs.handles:
                    nc.engines[h.engine].reg_load(h, t_meta[0:1, col : col + 1])
                return nc.snap(regs, min_val=lo, max_val=hi)

            g2s = [_loadsc(f"g2{k}", 2 * k, 0, T) for k in range(TPC)]
            lmaxs = [_loadsc(f"lmax{k}", 2 * k + 1, 1, T) for k in range(TPC)]

            for tile in range(TPC):
                lmax = lmaxs[tile]
                t_alpha = spool.tile([P, K], f32, tag=f"alpha{tile}")
                t_mask = spool.tile([P, T + 1], u8, tag=f"mask{tile}")
                t_tags = spool.tile([P, T], f32, tag=f"tags{tile}")
                nc.sync.dma_start(t_alpha, d_pot[tile, :, 0, :])
                nc.sync.dma_start(t_mask, d_mask[tile])
                nc.sync.dma_start(d_hist[tile, 0], t_alpha)

                # ---------------- forward ----------------
                # 2-step unrolled groups; GPSIMD (otherwise idle) adds
                # j-blocks [H, K) while DVE adds [0, H) and reduces both.
                # Chunked emit load + hist store (one dynamic DMA each per
                # group) keep SP bounds-check registers within budget.
                H = K // 2
                trCR3 = t_transCR.rearrange("p (j i) -> p j i", i=K)

                def fwd_step(t, t_emit, stage_slot, t_alpha=t_alpha,
                             t_mask=t_mask):
                    t_cand = iop.tile([P, K * K], f32, tag="cand")
                    cand3 = t_cand.rearrange("p (j i) -> p j i", i=K)
                    nc.vector.tensor_tensor(
                        out=cand3, in0=trCR3,
                        in1=t_alpha[:, None, :].broadcast_to([P, K, K]),
                        op=Op.add,
                    )
                    t_new = iop.tile([P, K], f32, tag="new")
                    nc.vector.tensor_reduce(
                        out=t_new, in_=cand3, axis=mybir.AxisListType.X, op=Op.max
                    )
                    nc.vector.tensor_tensor(out=t_new, in0=t_new, in1=t_emit, op=Op.add)
                    nc.vector.copy_predicated(
                        t_alpha, t_mask[:, ds(t, 1)].broadcast_to([P, K]), t_new
                    )
                    nc.vector.tensor_copy(stage_slot, t_alpha)

                g2 = g2s[tile]
                with tc.For_i(0, g2, 4, name=f"fwd4_{tile}") as i2:
                    t0 = nc.s_assert_within(i2 + 1, 1, T - 1,
                                            skip_runtime_assert=True)
                    t_emitC = iop.tile([P, 4 * K], f32, tag="emitC")
                    nc.sync.dma_start(
                        t_emitC.rearrange("p (u k) -> p u k", k=K),
                        d_pot[tile][:, ds(t0, 4), :])
                    t_stage = iop.tile([P, 4 * K], f32, tag="stage")
                    for u in range(4):
                        tu = nc.s_assert_within(i2 + (1 + u), 1, T,
                                                skip_runtime_assert=True)
                        fwd_step(tu, t_emitC[:, u * K:(u + 1) * K],
                                 t_stage[:, u * K:(u + 1) * K])
                    nc.sync.dma_start(
                        d_hist[tile][ds(t0, 4), :, :].transpose([1, 0, 2]),
                        t_stage.rearrange("p (u k) -> p u k", k=K))

                # ---------------- final scores / last tag ----------------
                t_best = spool.tile([P, 1], f32, tag=f"best{tile}")
                nc.vector.tensor_reduce(
                    out=t_best, in_=t_alpha, axis=mybir.AxisListType.X, op=Op.max
                )
                t_encf = iop.tile([P, K], f32, tag="encf")
                nc.vector.scalar_tensor_tensor(
                    out=t_encf, in0=t_alpha, scalar=t_best, in1=t_rev,
                    op0=Op.is_equal, op1=Op.mult,
                )
                t_lt = spool.tile([P, 1], f32, tag=f"lt{tile}")
                nc.vector.tensor_reduce(
                    out=t_lt, in_=t_encf, axis=mybir.AxisListType.X, op=Op.max
                )
                nc.sync.dma_start(d_best[tile], t_best)
                # prefill all tag columns with the (encoded) last tag
                nc.vector.tensor_copy(t_tags, t_lt.broadcast_to([P, T]))

                # ---------------- backward ----------------
                with tc.For_i(1, lmax) as i:
                    t = nc.s_assert_within(lmax - i, 1, T - 1)
                    t_hist = iop.tile([P, K], f32, tag="hist")
                    nc.sync.dma_start(t_hist, d_hist[tile][ds(t - 1, 1), :, :])
                    # one-hot of current tag: rev64[p,i] == enc  <=>  i == tag
                    t_selB = iop.tile([P, K], f32, tag="selB")
                    nc.vector.tensor_tensor(
                        out=t_selB, in0=t_rev,
                        in1=t_tags[:, ds(t, 1)].broadcast_to([P, K]),
                        op=Op.is_equal,
                    )
                    ps_selT = pp.tile([K, P], f32, tag="selT")
                    nc.tensor.transpose(ps_selT, t_selB, t_ident)
                    t_selT = iop.tile([K, P], f32, tag="selTs")
                    nc.scalar.copy(t_selT, ps_selT)
                    ps_gat = pp.tile([P, K], f32, tag="gat")
                    nc.tensor.matmul(
                        ps_gat, lhsT=t_selT, rhs=t_transT, start=True, stop=True
                    )
                    t_candb = iop.tile([P, K], f32, tag="candb")
                    nc.vector.tensor_tensor(
                        out=t_candb, in0=t_hist, in1=ps_gat, op=Op.add
                    )
                    t_bb = iop.tile([P, 1], f32, tag="bb")
                    nc.vector.tensor_reduce(
                        out=t_bb, in_=t_candb, axis=mybir.AxisListType.X, op=Op.max
                    )
                    t_encb = iop.tile([P, K], f32, tag="encb")
                    nc.vector.scalar_tensor_tensor(
                        out=t_encb, in0=t_candb, scalar=t_bb, in1=t_rev,
                        op0=Op.is_equal, op1=Op.mult,
                    )
                    t_penc = iop.tile([P, 1], f32, tag="penc")
                    nc.vector.tensor_reduce(
                        out=t_penc, in_=t_encb, axis=mybir.AxisListType.X, op=Op.max
                    )
                    nc.vector.copy_predicated(
                        t_tags[:, ds(t - 1, 1)], t_mask[:, ds(t, 1)], t_penc
                    )

                # decode: tag = 64 - enc, cast to int32
                t_tagsi = spool.tile([P, T], i32, tag=f"tagsi{tile}")
                nc.vector.tensor_scalar(
                    out=t_tagsi, in0=t_tags, scalar1=-1.0, scalar2=64.0,
                    op0=Op.mult, op1=Op.add,
                )
                nc.sync.dma_start(d_tags[tile], t_tagsi)

    return nc


def _prep_inputs(potentials, transitions, sequence_length):
    potentials = np.ascontiguousarray(np.asarray(potentials, dtype=np.float32))
    transitions = np.ascontiguousarray(np.asarray(transitions, dtype=np.float32))
    L = np.asarray(sequence_length).reshape(-1).astype(np.int64)

    order = np.argsort(-L, kind="stable")
    tiles = order.reshape(NTILES, P)
    Lmax_tile = L[tiles].max(axis=1).astype(np.int32)

    transT = np.ascontiguousarray(transitions.T)
    transCR = np.tile(transT.reshape(1, K * K), (P, 1)).astype(np.float32)
    ident = np.eye(P, dtype=np.float32)
    rev64 = np.tile((64.0 - np.arange(K, dtype=np.float32)).reshape(1, K), (P, 1))

    tvec = np.arange(T, dtype=np.int64)
    in_maps = []
    core_tiles = []
    for c in range(NCORES):
        ta, tb = c, NTILES - 1 - c
        sel = np.concatenate([tiles[ta], tiles[tb]])
        core_tiles.append((tiles[ta], tiles[tb]))
        pot_c = np.zeros((TPC, P, T + 1, K), dtype=np.float32)
        pot_c[:, :, :T, :] = potentials[sel].reshape(TPC, P, T, K)
        tvec1 = np.arange(T + 1, dtype=np.int64)
        mask_c = (tvec1[None, :] < np.minimum(L[sel][:, None], T)
                  ).astype(np.uint8).reshape(TPC, P, T + 1)
        def _g2(lm):
            return -((1 - int(lm)) // 4) * 4  # ceil((lm-1)/4)*4
        meta_c = np.array([[_g2(Lmax_tile[ta]), Lmax_tile[ta],
                            _g2(Lmax_tile[tb]), Lmax_tile[tb]]], dtype=np.int32)
        in_maps.append({
            "pot": pot_c,
            "maskf": np.ascontiguousarray(mask_c),
            "meta": meta_c,
            "transCR": transCR,
            "transT": transT,
            "ident": ident,
            "rev64": rev64,
        })
    return in_maps, core_tiles


def _run_hw(nc, in_maps, trace=False):
    from concourse.bass_utils import run_bass_kernel_spmd
    if not _CACHE.get("spilled"):
        _spill_excess_waits(nc)
        _CACHE["spilled"] = True
    res = run_bass_kernel_spmd(nc, in_maps, list(range(NCORES)), trace=trace)
    return res


def _run_sim(nc, in_maps, cores=None):
    from concourse import bass_interp
    outs = []
    for ci, im in enumerate(in_maps):
        if cores is not None and ci not in cores:
            outs.append(None)
            continue
        sim = bass_interp.CoreSim(nc)
        for k, v in im.items():
            sim.tensor(k)[:] = v
        sim.simulate()
        outs.append({
            "o_tags": np.array(sim.tensor("o_tags")),
            "o_best": np.array(sim.tensor("o_best")),
        })
    return outs


def kernel(potentials, transitions, sequence_length, _mode="hw", _cores=None,
           _trace=False):
    nc = _CACHE.get("nc")
    if nc is None:
        nc = _build_program()
        _CACHE["nc"] = nc
    in_maps, core_tiles = _prep_inputs(potentials, transitions, sequence_length)

    if _mode == "sim":
        results = _run_sim(nc, in_maps, cores=_cores)
    else:
        bres = _run_hw(nc, in_maps, trace=_trace)
        _CACHE["last_bres"] = bres
        results = bres.results

    tags = np.zeros((B, T), dtype=np.int32)
    best = np.zeros((B,), dtype=np.float32)
    for c, (rows_a, rows_b) in enumerate(core_tiles):
        r = results[c]
        if r is None:
            continue
        o_tags = np.asarray(r["o_tags"]).reshape(TPC, P, T)
        o_best = np.asarray(r["o_best"]).reshape(TPC, P)
        tags[rows_a] = o_tags[0]
        tags[rows_b] = o_tags[1]
        best[rows_a] = o_best[0]
        best[rows_b] = o_best[1]
    return tags, best
